# revision 11
# baseline (speedup 1.0000x reference)
"""Greedy CTC decoder on Trainium2 (Bass/Tile), sharded over 8 NeuronCores.

Input : emission [65536, 512] float32 (full, unsharded)
Output: (index [65536] int32, keep [65536] bool) matching the reference:
    index = argmax(emission, axis=-1)
    char  = index - 1 (blank 0 -> -1)
    keep  = (char != prev_char) & (char != -1)
          = (index != prev_index) & (index != 0),  prev of t=0 is a sentinel

Sharding: timestep axis T split across 8 cores (8192 rows each). Inside a
core, partition p owns the 64 consecutive timesteps p*64..p*64+63, so the
repeat-collapse comparison is a free-dim shift. The 64-step chunk boundary
(prev of j=0 lives on partition p-1) is resolved with one tiny SBUF->SBUF
DMA; the 7 shard boundaries are fixed on the host.
"""

import numpy as np

import concourse.bacc as bacc
import concourse.mybir as mybir
from concourse.tile import TileContext
from concourse.bass_utils import run_bass_kernel_spmd

N_CORES = 8
T_FULL = 65536
V = 512
P = 128
T_SHARD = T_FULL // N_CORES          # 8192
JPP = T_SHARD // P                   # 64 timesteps per partition
# chunk sizes (timesteps per partition per DMA): small first chunks so the
# DVE starts early, 2 MiB chunks later for full DMA efficiency
CHUNKS = [2, 2, 4] + [8] * 7
HALF = 32                            # keep-mask split point (after 6 chunks)
SENTINEL = 1000000.0                 # != any vocab index, exact in fp32

_prog_cache = {}


def _build():
    nc = bacc.Bacc(None, target_bir_lowering=False)

    em_h = nc.dram_tensor("emission", [T_SHARD, V], mybir.dt.float32,
                          kind="ExternalInput")
    idx_h = nc.dram_tensor("idx_out", [T_SHARD], mybir.dt.uint32,
                           kind="ExternalOutput")
    keep_h = nc.dram_tensor("keep_out", [T_SHARD], mybir.dt.uint8,
                            kind="ExternalOutput")

    # [T_SHARD, V] -> [P, JPP, V]: partition p holds rows p*JPP .. p*JPP+JPP-1
    em3 = em_h[:, :].rearrange("(p j) v -> p j v", p=P)
    idx_out2 = idx_h[:].rearrange("(p j) -> p j", p=P)
    keep_out2 = keep_h[:].rearrange("(p j) -> p j", p=P)

    with TileContext(nc) as tc:
        with (
            tc.tile_pool(name="io", bufs=4) as io_pool,
            tc.tile_pool(name="mx", bufs=4) as mx_pool,
            tc.tile_pool(name="acc", bufs=1) as acc_pool,
        ):
            # argmax ids for all 64 rows per partition, 8 slots per row
            # (max_index writes 8 indices; slot 0 is the argmax)
            idx8 = acc_pool.tile([P, JPP, 8], mybir.dt.uint32)
            idxc = acc_pool.tile([P, JPP], mybir.dt.uint32)
            neq = acc_pool.tile([P, JPP], mybir.dt.uint8)
            nz = acc_pool.tile([P, JPP], mybir.dt.uint8)
            keep = acc_pool.tile([P, JPP], mybir.dt.uint8)

            def keep_phase(lo, hi):
                """Repeat-collapse for columns [lo, hi) on GpSimd (DVE stays
                on max_index). Column 0 is deferred to the caller."""
                v = nc.vector
                # compact argmax (slot 0 of each 8-group) to contiguous u32
                v.tensor_copy(idxc[:, lo:hi], idx8[:, lo:hi, 0])
                lo1 = max(lo, 1)  # column 0 needs the cross-partition prev
                v.tensor_tensor(out=neq[:, lo1:hi], in0=idxc[:, lo1:hi],
                                in1=idxc[:, lo1 - 1:hi - 1],
                                op=mybir.AluOpType.not_equal)
                v.tensor_scalar(out=nz[:, lo:hi], in0=idxc[:, lo:hi],
                                scalar1=0.0, scalar2=None,
                                op0=mybir.AluOpType.not_equal)
                v.tensor_tensor(out=keep[:, lo1:hi], in0=neq[:, lo1:hi],
                                in1=nz[:, lo1:hi], op=mybir.AluOpType.mult)
                # outputs ride the ACT HWDGE ring: the sync ring's FIFO is
                # busy with multi-MiB input chunks
                nc.scalar.dma_start(out=idx_out2[:, lo:hi], in_=idxc[:, lo:hi])
                nc.scalar.dma_start(out=keep_out2[:, lo1:hi],
                                    in_=keep[:, lo1:hi])

            j = 0
            for c, n in enumerate(CHUNKS):
                tile = io_pool.tile([P, n, V], mybir.dt.float32)
                # first chunk on the (idle) ACT ring so it lands while the
                # sync ring is still ramping chunk 1
                eng = nc.scalar if c == 0 else nc.sync
                eng.dma_start(out=tile[:, :, :], in_=em3[:, j:j + n, :])
                # one reduce for all n rows' maxes (552ns/row vs 608 for
                # per-row InstMax)
                rowmax = mx_pool.tile([P, n], mybir.dt.float32)
                nc.vector.tensor_reduce(out=rowmax[:, :], in_=tile[:, :, :],
                                        axis=mybir.AxisListType.X,
                                        op=mybir.AluOpType.max)
                for k in range(n):
                    nc.vector.max_index(
                        out=idx8[:, j + k, :],
                        in_max=rowmax[:, k:k + 1].broadcast_to((P, 8)),
                        in_values=tile[:, k, :])
                j += n
                if j == HALF:
                    keep_phase(0, HALF)

            keep_phase(HALF, JPP)
            # column 0 of each partition (t % 64 == 0) is resolved on the
            # host: it needs the previous partition/shard's last index, and
            # a 128-byte cross-partition DMA costs ~3us of tail latency here

    nc.compile()
    return nc


def _get_prog():
    if "nc" not in _prog_cache:
        _prog_cache["nc"] = _build()
    return _prog_cache["nc"]


def run_sharded(emission: np.ndarray, **spmd_kwargs):
    """Run the SPMD kernel; returns (idx int32 [T], keep bool [T], results)."""
    emission = np.ascontiguousarray(np.asarray(emission, dtype=np.float32))
    assert emission.shape == (T_FULL, V), emission.shape
    nc = _get_prog()
    in_maps = [
        {"emission": np.ascontiguousarray(emission[c * T_SHARD:(c + 1) * T_SHARD])}
        for c in range(N_CORES)
    ]
    res = run_bass_kernel_spmd(nc, in_maps, list(range(N_CORES)), **spmd_kwargs)
    idx = np.concatenate([res.results[c]["idx_out"] for c in range(N_CORES)])
    keep = np.concatenate([res.results[c]["keep_out"] for c in range(N_CORES)])
    idx = idx.astype(np.int32, copy=False)
    keep = keep.astype(bool, copy=False)
    # boundary exchange: the device leaves every 64-step chunk's first
    # timestep unresolved (cross-partition/shard prev); fix them all here
    b = np.arange(64, T_FULL, 64)
    keep[b] = (idx[b] != idx[b - 1]) & (idx[b] != 0)
    keep[0] = idx[0] != 0
    return idx, keep, res


def kernel(emission: np.ndarray):
    idx, keep, _ = run_sharded(emission)
    return idx, keep


# revision 12
# speedup vs baseline: 1.0070x; 1.0070x over previous
"""Greedy CTC decoder on Trainium2 (Bass/Tile), sharded over 8 NeuronCores.

Input : emission [65536, 512] float32 (full, unsharded)
Output: (index [65536] int32, keep [65536] bool) matching the reference:
    index = argmax(emission, axis=-1)
    char  = index - 1 (blank 0 -> -1)
    keep  = (char != prev_char) & (char != -1)
          = (index != prev_index) & (index != 0),  prev of t=0 is a sentinel

Sharding: timestep axis T split across 8 cores (8192 rows each). Inside a
core, partition p owns the 64 consecutive timesteps p*64..p*64+63, so the
repeat-collapse comparison is a free-dim shift. The 64-step chunk boundary
(prev of j=0 lives on partition p-1) is resolved with one tiny SBUF->SBUF
DMA; the 7 shard boundaries are fixed on the host.
"""

import numpy as np

import concourse.bacc as bacc
import concourse.mybir as mybir
from concourse.tile import TileContext
from concourse.bass_utils import run_bass_kernel_spmd

N_CORES = 8
T_FULL = 65536
V = 512
P = 128
T_SHARD = T_FULL // N_CORES          # 8192
JPP = T_SHARD // P                   # 64 timesteps per partition
# chunk sizes (timesteps per partition per DMA): small first chunks so the
# DVE starts early, 2 MiB chunks later for full DMA efficiency
CHUNKS = [2, 2, 4] + [8] * 7
HALF = 32                            # keep-mask split point (after 6 chunks)
SENTINEL = 1000000.0                 # != any vocab index, exact in fp32

_prog_cache = {}


def _build():
    nc = bacc.Bacc(None, target_bir_lowering=False)

    em_h = nc.dram_tensor("emission", [T_SHARD, V], mybir.dt.float32,
                          kind="ExternalInput")
    idx_h = nc.dram_tensor("idx_out", [T_SHARD], mybir.dt.uint32,
                           kind="ExternalOutput")
    keep_h = nc.dram_tensor("keep_out", [T_SHARD], mybir.dt.uint8,
                            kind="ExternalOutput")

    # [T_SHARD, V] -> [P, JPP, V]: partition p holds rows p*JPP .. p*JPP+JPP-1
    em3 = em_h[:, :].rearrange("(p j) v -> p j v", p=P)
    idx_out2 = idx_h[:].rearrange("(p j) -> p j", p=P)
    keep_out2 = keep_h[:].rearrange("(p j) -> p j", p=P)

    with TileContext(nc) as tc:
        with (
            tc.tile_pool(name="io", bufs=4) as io_pool,
            tc.tile_pool(name="mx", bufs=4) as mx_pool,
            tc.tile_pool(name="acc", bufs=1) as acc_pool,
        ):
            # argmax ids for all 64 rows per partition, 8 slots per row
            # (max_index writes 8 indices; slot 0 is the argmax)
            idx8 = acc_pool.tile([P, JPP, 8], mybir.dt.uint32)
            idxc = acc_pool.tile([P, JPP], mybir.dt.uint32)
            neq = acc_pool.tile([P, JPP], mybir.dt.uint8)
            nz = acc_pool.tile([P, JPP], mybir.dt.uint8)
            keep = acc_pool.tile([P, JPP], mybir.dt.uint8)

            def keep_phase(lo, hi):
                """Repeat-collapse for columns [lo, hi) on GpSimd (DVE stays
                on max_index). Column 0 is deferred to the caller."""
                v = nc.vector
                # compact argmax (slot 0 of each 8-group) to contiguous u32
                v.tensor_copy(idxc[:, lo:hi], idx8[:, lo:hi, 0])
                lo1 = max(lo, 1)  # column 0 needs the cross-partition prev
                v.tensor_tensor(out=neq[:, lo1:hi], in0=idxc[:, lo1:hi],
                                in1=idxc[:, lo1 - 1:hi - 1],
                                op=mybir.AluOpType.not_equal)
                v.tensor_scalar(out=nz[:, lo:hi], in0=idxc[:, lo:hi],
                                scalar1=0.0, scalar2=None,
                                op0=mybir.AluOpType.not_equal)
                v.tensor_tensor(out=keep[:, lo1:hi], in0=neq[:, lo1:hi],
                                in1=nz[:, lo1:hi], op=mybir.AluOpType.mult)
                nc.sync.dma_start(out=idx_out2[:, lo:hi], in_=idxc[:, lo:hi])
                nc.sync.dma_start(out=keep_out2[:, lo1:hi],
                                  in_=keep[:, lo1:hi])

            j = 0
            for c, n in enumerate(CHUNKS):
                tile = io_pool.tile([P, n, V], mybir.dt.float32)
                nc.sync.dma_start(out=tile[:, :, :], in_=em3[:, j:j + n, :])
                # one reduce for all n rows' maxes (552ns/row vs 608 for
                # per-row InstMax)
                rowmax = mx_pool.tile([P, n], mybir.dt.float32)
                nc.vector.tensor_reduce(out=rowmax[:, :], in_=tile[:, :, :],
                                        axis=mybir.AxisListType.X,
                                        op=mybir.AluOpType.max)
                for k in range(n):
                    nc.vector.max_index(
                        out=idx8[:, j + k, :],
                        in_max=rowmax[:, k:k + 1].broadcast_to((P, 8)),
                        in_values=tile[:, k, :])
                j += n
                if j == HALF:
                    keep_phase(0, HALF)

            keep_phase(HALF, JPP)
            # column 0 of each partition (t % 64 == 0) is resolved on the
            # host: it needs the previous partition/shard's last index, and
            # a 128-byte cross-partition DMA costs ~3us of tail latency here

    nc.compile()
    return nc


def _get_prog():
    if "nc" not in _prog_cache:
        _prog_cache["nc"] = _build()
    return _prog_cache["nc"]


def run_sharded(emission: np.ndarray, **spmd_kwargs):
    """Run the SPMD kernel; returns (idx int32 [T], keep bool [T], results)."""
    emission = np.ascontiguousarray(np.asarray(emission, dtype=np.float32))
    assert emission.shape == (T_FULL, V), emission.shape
    nc = _get_prog()
    in_maps = [
        {"emission": np.ascontiguousarray(emission[c * T_SHARD:(c + 1) * T_SHARD])}
        for c in range(N_CORES)
    ]
    res = run_bass_kernel_spmd(nc, in_maps, list(range(N_CORES)), **spmd_kwargs)
    idx = np.concatenate([res.results[c]["idx_out"] for c in range(N_CORES)])
    keep = np.concatenate([res.results[c]["keep_out"] for c in range(N_CORES)])
    idx = idx.astype(np.int32, copy=False)
    keep = keep.astype(bool, copy=False)
    # boundary exchange: the device leaves every 64-step chunk's first
    # timestep unresolved (cross-partition/shard prev); fix them all here
    b = np.arange(64, T_FULL, 64)
    keep[b] = (idx[b] != idx[b - 1]) & (idx[b] != 0)
    keep[0] = idx[0] != 0
    return idx, keep, res


def kernel(emission: np.ndarray):
    idx, keep, _ = run_sharded(emission)
    return idx, keep


# revision 13
# speedup vs baseline: 1.0704x; 1.0629x over previous
"""Greedy CTC decoder on Trainium2 (Bass/Tile), sharded over 8 NeuronCores.

Input : emission [65536, 512] float32 (full, unsharded)
Output: (index [65536] int32, keep [65536] bool) matching the reference:
    index = argmax(emission, axis=-1)
    char  = index - 1 (blank 0 -> -1)
    keep  = (char != prev_char) & (char != -1)
          = (index != prev_index) & (index != 0),  prev of t=0 is a sentinel

Sharding: timestep axis T split across 8 cores (8192 rows each). Inside a
core, partition p owns the 64 consecutive timesteps p*64..p*64+63, so the
repeat-collapse comparison is a free-dim shift. The 64-step chunk boundary
(prev of j=0 lives on partition p-1) is resolved with one tiny SBUF->SBUF
DMA; the 7 shard boundaries are fixed on the host.
"""

import numpy as np

import concourse.bacc as bacc
import concourse.mybir as mybir
from concourse.tile import TileContext
from concourse.bass_utils import run_bass_kernel_spmd

N_CORES = 8
T_FULL = 65536
V = 512
P = 128
T_SHARD = T_FULL // N_CORES          # 8192
JPP = T_SHARD // P                   # 64 timesteps per partition
# chunk sizes (timesteps per partition per DMA): small first chunks so the
# DVE starts early, 2 MiB chunks later for full DMA efficiency
CHUNKS = [2, 2, 4] + [8] * 7
HALF = 32                            # keep-mask split point (after 6 chunks)
SENTINEL = 1000000.0                 # != any vocab index, exact in fp32

_prog_cache = {}


def _build():
    nc = bacc.Bacc(None, target_bir_lowering=False)

    em_h = nc.dram_tensor("emission", [T_SHARD, V], mybir.dt.float32,
                          kind="ExternalInput")
    idx_h = nc.dram_tensor("idx_out", [T_SHARD], mybir.dt.uint32,
                           kind="ExternalOutput")
    keep_h = nc.dram_tensor("keep_out", [T_SHARD], mybir.dt.uint8,
                            kind="ExternalOutput")

    # [T_SHARD, V] -> [P, JPP, V]: partition p holds rows p*JPP .. p*JPP+JPP-1
    em3 = em_h[:, :].rearrange("(p j) v -> p j v", p=P)
    idx_out2 = idx_h[:].rearrange("(p j) -> p j", p=P)
    keep_out2 = keep_h[:].rearrange("(p j) -> p j", p=P)

    with TileContext(nc) as tc:
        with (
            tc.tile_pool(name="io", bufs=4) as io_pool,
            tc.tile_pool(name="mx", bufs=4) as mx_pool,
            tc.tile_pool(name="acc", bufs=1) as acc_pool,
        ):
            # raw argmax stream-indices: for 8-row chunks one FIND_INDEX8
            # searches all 8 rows at once (needle k = row k's max), so the
            # value is (k*512 + argmax). Cross-row bitwise-equal collisions
            # are detected host-side via the k bits and repaired there.
            idxr = acc_pool.tile([P, JPP], mybir.dt.uint32)
            small8 = acc_pool.tile([P, 8, 8], mybir.dt.uint32)
            idxc = acc_pool.tile([P, JPP], mybir.dt.uint32)
            offs = acc_pool.tile([P, JPP], mybir.dt.uint32)
            offs_np = np.zeros((P, JPP), dtype=np.uint32)
            for jj in range(8, JPP):
                offs_np[:, jj] = (jj % 8) * V
            offs_dram = nc.inline_tensor(offs_np, name="offs_const")
            nc.sync.dma_start(out=offs[:, :], in_=offs_dram[:, :])
            neq = acc_pool.tile([P, JPP], mybir.dt.uint8)
            nz = acc_pool.tile([P, JPP], mybir.dt.uint8)
            keep = acc_pool.tile([P, JPP], mybir.dt.uint8)

            def keep_phase(lo, hi):
                """Repeat-collapse for columns [lo, hi) on GpSimd (DVE stays
                on max_index). Column 0 is deferred to the caller."""
                v = nc.vector
                # strip the within-chunk row offset: idxc = idxr - k*512
                v.tensor_tensor(out=idxc[:, lo:hi], in0=idxr[:, lo:hi],
                                in1=offs[:, lo:hi],
                                op=mybir.AluOpType.subtract)
                lo1 = max(lo, 1)  # column 0 needs the cross-partition prev
                v.tensor_tensor(out=neq[:, lo1:hi], in0=idxc[:, lo1:hi],
                                in1=idxc[:, lo1 - 1:hi - 1],
                                op=mybir.AluOpType.not_equal)
                v.tensor_scalar(out=nz[:, lo:hi], in0=idxc[:, lo:hi],
                                scalar1=0.0, scalar2=None,
                                op0=mybir.AluOpType.not_equal)
                v.tensor_tensor(out=keep[:, lo1:hi], in0=neq[:, lo1:hi],
                                in1=nz[:, lo1:hi], op=mybir.AluOpType.mult)
                nc.sync.dma_start(out=idx_out2[:, lo:hi], in_=idxr[:, lo:hi])
                nc.sync.dma_start(out=keep_out2[:, lo1:hi],
                                  in_=keep[:, lo1:hi])

            j = 0
            for c, n in enumerate(CHUNKS):
                tile = io_pool.tile([P, n, V], mybir.dt.float32)
                nc.sync.dma_start(out=tile[:, :, :], in_=em3[:, j:j + n, :])
                # one reduce for all n rows' maxes (552ns/row vs 608 for
                # per-row InstMax)
                rowmax = mx_pool.tile([P, 8], mybir.dt.float32)
                nc.vector.tensor_reduce(out=rowmax[:, 0:n], in_=tile[:, :, :],
                                        axis=mybir.AxisListType.X,
                                        op=mybir.AluOpType.max)
                if n == 8:
                    # one FIND_INDEX8 for all 8 rows: needles are the 8 row
                    # maxes, scanned over the whole 4096-element chunk
                    nc.vector.max_index(
                        out=idxr[:, j:j + 8],
                        in_max=rowmax[:, :],
                        in_values=tile[:, :, :].rearrange("p a v -> p (a v)"))
                else:
                    for k in range(n):
                        nc.vector.max_index(
                            out=small8[:, j + k, :],
                            in_max=rowmax[:, k:k + 1].broadcast_to((P, 8)),
                            in_values=tile[:, k, :])
                j += n
                if j == 8:
                    # compact the per-row results of the small head chunks
                    nc.vector.tensor_copy(idxr[:, 0:8], small8[:, :, 0])
                if j == HALF:
                    keep_phase(0, HALF)

            keep_phase(HALF, JPP)
            # column 0 of each partition (t % 64 == 0) is resolved on the
            # host: it needs the previous partition/shard's last index, and
            # a 128-byte cross-partition DMA costs ~3us of tail latency here

    nc.compile()
    return nc


def _get_prog():
    if "nc" not in _prog_cache:
        _prog_cache["nc"] = _build()
    return _prog_cache["nc"]


def run_sharded(emission: np.ndarray, **spmd_kwargs):
    """Run the SPMD kernel; returns (idx int32 [T], keep bool [T], results)."""
    emission = np.ascontiguousarray(np.asarray(emission, dtype=np.float32))
    assert emission.shape == (T_FULL, V), emission.shape
    nc = _get_prog()
    in_maps = [
        {"emission": np.ascontiguousarray(emission[c * T_SHARD:(c + 1) * T_SHARD])}
        for c in range(N_CORES)
    ]
    res = run_bass_kernel_spmd(nc, in_maps, list(range(N_CORES)), **spmd_kwargs)
    raw = np.concatenate([res.results[c]["idx_out"] for c in range(N_CORES)])
    keep = np.concatenate([res.results[c]["keep_out"] for c in range(N_CORES)])
    idx = (raw & (V - 1)).astype(np.int32)
    keep = keep.astype(bool, copy=False)
    # detect cross-row collisions in the batched FIND_INDEX8: the needle
    # matched in the wrong row's segment
    j_arr = np.arange(T_FULL) % JPP
    expected = np.where(j_arr < 8, 0, j_arr % 8).astype(np.uint32)
    corrupt = np.nonzero((raw >> 9) != expected)[0]
    for t in corrupt:
        idx[t] = int(np.argmax(emission[t]))
    for t0 in corrupt:
        for t in (t0, t0 + 1):
            if t < T_FULL:
                keep[t] = bool((idx[t] != (idx[t - 1] if t else -1))
                               and (idx[t] != 0))
    # boundary exchange: the device leaves every 64-step chunk's first
    # timestep unresolved (cross-partition/shard prev); fix them all here
    b = np.arange(64, T_FULL, 64)
    keep[b] = (idx[b] != idx[b - 1]) & (idx[b] != 0)
    keep[0] = idx[0] != 0
    return idx, keep, res


def kernel(emission: np.ndarray):
    idx, keep, _ = run_sharded(emission)
    return idx, keep
